# revision 18
# baseline (speedup 1.0000x reference)
"""Trainium2 Bass kernel for nn_Attention (dual-softmax linear attention).

v6: fp8 DoubleRow matmuls for the three large projections (kv-proj, q-proj,
final projection) at ~1.8x the fp16 per-matmul rate, with a centering scheme
that keeps rel_err at fp16 levels (~5e-4 in numpy sim):

  - Host passes x pre-transposed AND centered: xt8 = fp8(2x-1) [D, S].
    No DMA/PE transposes on device at all.
  - Wq8 = fp8(64*Wq), Wkv8 = fp8(64*Wkv) with Wkv columns PERMUTED so all
    k-columns come first (cols 0:1024 = k of heads 0..15), then v-columns.
  - q logits: q = (xt8.T @ Wq8)/128 + cst_q, cst_q = 0.5*colsum(Wq) passed
    from host in fp32 and applied as the per-partition activation bias of
    the Exp evac (exact restoration of the x-mean term).
  - k logits: k~ = (xt8.T @ Wk8)/128 WITHOUT the constant — a per-column
    constant on k cancels in the k-softmax normalization.
  - v: v~ = (xt8.T @ Wv8)/128 without its constant; the v-constant's entire
    contribution to y collapses (softmax weights sum to 1) to a per-output-
    column constant added on the HOST in fp32:
        kappa[c] = SCALE * sum_e 0.5*colsum(Wv)[e] * Wlin16[e, c]
    Removing the dominant rank-1 component of ctx from the device path is
    what makes fp8 quantization of the eq8/W28 phase-B operands harmless.
  - y output in fp16; host adds kappa + blin in fp32.

Engine-load structure (v6):
  - ctx and colsum accumulate in PSUM across ALL chunks (no DVE adds): a
    ones-column appended to each v-tile folds colsum into the ctx matmul
    (out[d, 0:128] = ctx[d, e], out[d, 128] = colsum[d]), one matmul +
    one ldweights per (chunk, t, j). Three packed PSUM banks hold all 8
    head-pair accumulators for the whole of phase A.
  - ctx lands [d, e]; the finalize transposes the two 64x64 diagonal
    blocks to bdt16 [e, d] via DVE 32x32 stream-transposes (PSUM -> SBUF).
  - eq8 = fp8(64*eqn) via a single DVE multiply (rr = 64/rowsum).

Sharding: data-parallel over batch B=8 -> one batch element per NeuronCore.
"""

import numpy as np

import concourse.bass as bass
import concourse.mybir as mybir
from concourse import bacc
from concourse.tile import TileContext
from concourse.masks import make_identity

F32 = mybir.dt.float32
F16 = mybir.dt.float16
F8 = mybir.dt.float8e4
AF = mybir.ActivationFunctionType
DR = mybir.MatmulPerfMode.DoubleRow

S, D = 4096, 1024
H, DH = 16, 64
DM = H * DH  # 1024
B = 8
SCALE = DH ** (-0.5)
C = 17  # W2 fixed-point exponent

P = 128          # partitions
NB = 512         # moving free-dim tile
ND = D // P      # 8 d-tiles
NDP = ND // 2    # 4 d-tile pairs (DoubleRow)
NJ = DM // P     # 8 dout-tiles
CW = 132         # ctx psum region pitch (129 used + pad)


def build_nc(s_len=S):
    sc = s_len // NB
    nc = bacc.Bacc(None, target_bir_lowering=False)

    xt_in = nc.declare_dram_parameter("xt", [D, s_len], F8, isOutput=False)
    wq_in = nc.declare_dram_parameter("Wq", [D, DM], F8, isOutput=False)
    wkv_in = nc.declare_dram_parameter("Wkv", [D, 2 * DM], F8, isOutput=False)
    wlin_in = nc.declare_dram_parameter("Wlin", [DM, DM], F16, isOutput=False)
    cstq_in = nc.declare_dram_parameter("cstq", [P, NJ], F32, isOutput=False)
    y_out = nc.declare_dram_parameter("y", [s_len, DM], F16, isOutput=True)

    with TileContext(nc) as tc:
        from contextlib import ExitStack

        with ExitStack() as stk:
            consts = stk.enter_context(tc.tile_pool(name="consts", bufs=1))
            wbig = stk.enter_context(tc.tile_pool(name="wbig", bufs=1))

            ident = consts.tile([P, P], F16, tag="ident")
            make_identity(nc, ident)
            blkones = consts.tile([P, P], F16, tag="blkones")
            nc.vector.memset(blkones, 0.0)
            nc.vector.memset(blkones[0:64, 0:64], 1.0 / 64)
            nc.vector.memset(blkones[64:128, 64:128], 1.0 / 64)
            zeros396 = consts.tile([P, 3 * CW], F16, tag="zeros396")
            nc.vector.memset(zeros396, 0.0)
            cstq_sb = consts.tile([P, NJ], F32, tag="cstq")
            nc.sync.dma_start(out=cstq_sb, in_=cstq_in[0:P, 0:NJ])

            # blockdiag ctx^T staging tiles (off-diag zeros set once)
            bdt_tiles = []
            for j in range(NJ):
                bdt = consts.tile([P, P], F16, tag=f"bdt{j}", name=f"bdt{j}")
                nc.vector.memset(bdt, 0.0)
                bdt_tiles.append(bdt)

            wkv_sb = wbig.tile([P, ND, 2 * DM], F8, tag="wkv", name="wkv")
            wq_sb = wbig.tile([P, ND, DM], F8, tag="wq", name="wq")
            wlin_sb = wbig.tile([P, ND, DM], F16, tag="wlin", name="wlin")
            w28_sb = wbig.tile([P, NJ, DM], F8, tag="w28", name="w28")

            xt_pool = stk.enter_context(tc.tile_pool(name="xt", bufs=3))
            ek_pool = stk.enter_context(tc.tile_pool(name="ek", bufs=1))
            vt_pool = stk.enter_context(tc.tile_pool(name="vt", bufs=1))
            eq_pool = stk.enter_context(tc.tile_pool(name="eq", bufs=1))
            rr_pool = stk.enter_context(tc.tile_pool(name="rr", bufs=2))
            e8_pool = stk.enter_context(tc.tile_pool(name="e8", bufs=1))
            e8_res = [None] * sc
            rcs_tiles = [None] * NJ

            # v tiles carry a ones column at [:, :, P] so colsum folds into
            # the ctx matmul; set it once per buffer here
            for par in range(2):
                for t in range(4):
                    vt = vt_pool.tile([P, NJ, P + 1], F8,
                                      tag=f"v{par}_{t}", name=f"v{par}_{t}")
                    nc.vector.memset(vt[:, :, P:P + 1], 1.0)

            def load_xt(c, xt_t):
                for jd in range(ND):
                    nc.sync.dma_start(
                        out=xt_t[:, jd, :],
                        in_=xt_in[jd * P:(jd + 1) * P, c * NB:(c + 1) * NB],
                    )

            # ---------------- phase A ----------------
            with (
                tc.tile_pool(name="kvp", bufs=2, space="PSUM") as kvp_pool,
                tc.tile_pool(name="ctxp", bufs=1, space="PSUM") as ctxp_pool,
                tc.tile_pool(name="qp", bufs=2, space="PSUM") as qp_pool,
            ):
                # packed ctx+colsum accumulators: 3 head-pairs per bank,
                # alive across all of phase A
                ctxg = [
                    ctxp_pool.tile([P, 3 * CW], F32, tag=f"ctxg{g}",
                                   name=f"ctxg{g}")
                    for g in range(3)
                ]

                def ctx_region(j):
                    return ctxg[j // 3], (j % 3) * CW

                # initialize each ctx bank with a zero matmul carrying the
                # only start=True (start clears has_written BANK-wide on HW);
                # all real ctx matmuls then accumulate with start=False
                for w in range(18):
                    nc.tensor.matmul(
                        ctxg[w % 3], blkones, zeros396, start=True, stop=True,
                        skip_group_check=True,
                    )

                for c in range(sc):
                    xt_t = xt_pool.tile([P, ND, NB], F8, tag="xt")
                    load_xt(c, xt_t)
                    if c == 0:
                        # weight DMAs split across the gpsimd/scalar trigger
                        # queues (xt rides the sync queue)
                        for half in range(2):
                            for n in range(2):
                                lo = half * DM + n * NB
                                for jd in range(ND):
                                    eng = (nc.gpsimd if jd % 2 == 0
                                           else nc.scalar)
                                    eng.dma_start(
                                        out=wkv_sb[:, jd, lo:lo + NB],
                                        in_=wkv_in[jd * P:(jd + 1) * P,
                                                   lo:lo + NB],
                                    )
                        for jd in range(ND):
                            eng = nc.gpsimd if jd % 2 == 0 else nc.scalar
                            eng.dma_start(
                                out=wq_sb[:, jd, :],
                                in_=wq_in[jd * P:(jd + 1) * P, :],
                            )
                        for jd in range(ND):
                            eng = nc.gpsimd if jd % 2 == 0 else nc.scalar
                            eng.dma_start(
                                out=wlin_sb[:, jd, :],
                                in_=wlin_in[jd * P:(jd + 1) * P, :],
                            )

                    # kv projection (fp8 DoubleRow, K=256 per matmul)
                    ek_tiles = [None] * 4
                    v_tiles = [None] * 4
                    for t in range(4):
                        ek_tiles[t] = ek_pool.tile(
                            [P, DM], F8, tag=f"ek{c % 2}_{t}",
                            name=f"ek{c}_{t}"
                        )
                        v_tiles[t] = vt_pool.tile(
                            [P, NJ, P + 1], F8, tag=f"v{c % 2}_{t}",
                            name=f"vt{c}_{t}"
                        )
                    # k-half for all t first (weights stream k-cols first),
                    # then v-half; v evacs ride DVE to offload ScalarE
                    for half in range(2):
                        for n in range(2):
                            for t in range(4):
                                kvps = kvp_pool.tile([P, NB], F32, tag="kvp")
                                for jp in range(NDP):
                                    nc.tensor.matmul(
                                        kvps,
                                        xt_t[:, 2 * jp:2 * jp + 2,
                                             t * P:(t + 1) * P],
                                        wkv_sb[:, 2 * jp:2 * jp + 2,
                                               half * DM + n * NB:
                                               half * DM + (n + 1) * NB],
                                        start=(jp == 0),
                                        stop=(jp == NDP - 1),
                                        perf_mode=DR,
                                    )
                                if half == 0:
                                    nc.scalar.activation(
                                        ek_tiles[t][:, n * NB:(n + 1) * NB],
                                        kvps, AF.Exp, scale=1.0 / 128,
                                    )
                                else:
                                    nc.vector.tensor_scalar_mul(
                                        out=v_tiles[t][:, 4 * n:4 * n + 4, 0:P],
                                        in0=kvps.rearrange(
                                            "p (j e) -> p j e", j=4),
                                        scalar1=1.0 / 128,
                                    )

                    def ctx_block():
                        # ctx+colsum accumulate in PSUM across all chunks
                        for j in range(NJ):
                            cg, base = ctx_region(j)
                            for t in range(4):
                                # start=True clears has_written BANK-wide on
                                # HW, so only the bank's first matmul may
                                # carry it; co-tenant regions overwrite on
                                # virgin has_written instead
                                nc.tensor.matmul(
                                    cg[:, base:base + P + 1],
                                    ek_tiles[t][:, j * P:(j + 1) * P],
                                    v_tiles[t][:, j, 0:P + 1],
                                    start=False,
                                    stop=False,
                                    skip_group_check=True,
                                )

                    if c == sc - 1:
                        ctx_block()
                        # finalize inline, ahead of q/rowsum so their DVE
                        # backlog doesn't delay W28 (phase B's input)
                        bsrc_tiles = [None] * NJ
                        for j in range(NJ):
                            cg, base = ctx_region(j)
                            bsrc = consts.tile(
                                [P, P], F16, tag=f"bsrc{j}", name=f"bsrc{j}"
                            )
                            for db in range(2):
                                o = 64 * db
                                nc.scalar.activation(
                                    bsrc[o:o + 64, o:o + 64],
                                    cg[o:o + 64, base + o:base + o + 64],
                                    AF.Copy,
                                )
                            bsrc_tiles[j] = bsrc
                            cs_sb = consts.tile(
                                [P, 1], F32, tag=f"cs{j}", name=f"cs{j}"
                            )
                            nc.scalar.activation(
                                cs_sb, cg[:, base + P:base + P + 1], AF.Copy
                            )
                            rcs = consts.tile([P, 1], F32, tag=f"rcs{j}")
                            nc.vector.reciprocal_approx_fast(
                                out=rcs, in_=cs_sb
                            )
                            rcs_tiles[j] = rcs
                        for j in range(NJ):
                            trps = qp_pool.tile([P, NB], F32, tag="qp")
                            nc.tensor.matmul(
                                trps[:, 0:P], bsrc_tiles[j], ident
                            )
                            for db in range(2):
                                o = 64 * db
                                nc.scalar.activation(
                                    bdt_tiles[j][o:o + 64, o:o + 64],
                                    trps[o:o + 64, o:o + 64],
                                    AF.Copy,
                                )
                        for j in range(NJ):
                            for n in range(2):
                                w2ps = qp_pool.tile([P, NB], F32, tag="qp")
                                nc.tensor.matmul(
                                    w2ps,
                                    bdt_tiles[j],
                                    wlin_sb[:, j, n * NB:(n + 1) * NB],
                                )
                                nc.vector.tensor_scalar(
                                    out=w28_sb[:, j, n * NB:(n + 1) * NB],
                                    in0=w2ps,
                                    scalar1=rcs_tiles[j],
                                    scalar2=SCALE * float(2.0 ** C),
                                    op0=mybir.AluOpType.mult,
                                    op1=mybir.AluOpType.mult,
                                )

                    # q projection (fp8 DoubleRow) -> eq16
                    e8_t = e8_pool.tile([P, NJ, NB], F8, tag=f"e8_{c}")
                    e8_res[c] = e8_t
                    eq16_tiles = [None] * NJ
                    for j in range(NJ):
                        qps = qp_pool.tile([P, NB], F32, tag="qp")
                        for jp in range(NDP):
                            nc.tensor.matmul(
                                qps,
                                wq_sb[:, 2 * jp:2 * jp + 2, j * P:(j + 1) * P],
                                xt_t[:, 2 * jp:2 * jp + 2, :],
                                start=(jp == 0),
                                stop=(jp == NDP - 1),
                                perf_mode=DR,
                            )
                        eq16 = eq_pool.tile([P, NB], F16, tag=f"eq{j}")
                        nc.scalar.activation(
                            eq16, qps, AF.Exp,
                            scale=1.0 / 128, bias=cstq_sb[:, j:j + 1],
                        )
                        eq16_tiles[j] = eq16

                    # rowsum (1/64-blockones matmul) -> rr -> eq8 = eq16*rr
                    for j in range(NJ):
                        eq16 = eq16_tiles[j]
                        rsps = qp_pool.tile([P, NB], F32, tag="qp")
                        nc.tensor.matmul(rsps, blkones, eq16)
                        rr = rr_pool.tile([P, NB], F32, tag="rr")
                        nc.vector.reciprocal_approx_fast(out=rr, in_=rsps)
                        nc.vector.tensor_mul(e8_t[:, j, :], eq16, rr)

                    if c < sc - 1:
                        ctx_block()

            y_pool = stk.enter_context(tc.tile_pool(name="ysb", bufs=3))

            # ---------------- phase B: y = eq8.T @ W28 (fp8 DoubleRow)
            with tc.tile_pool(name="yp", bufs=4, space="PSUM") as yp_pool:
                for c in range(sc):
                    for t in range(4):
                        yps = yp_pool.tile([P, DM], F32, tag="yp")
                        for n in range(2):
                            for jp in range(NJ // 2):
                                nc.tensor.matmul(
                                    yps[:, n * NB:(n + 1) * NB],
                                    e8_res[c][:, 2 * jp:2 * jp + 2,
                                              t * P:(t + 1) * P],
                                    w28_sb[:, 2 * jp:2 * jp + 2,
                                           n * NB:(n + 1) * NB],
                                    start=(jp == 0),
                                    stop=(jp == NJ // 2 - 1),
                                    perf_mode=DR,
                                )
                        ysb = y_pool.tile([P, DM], F16, tag="ysb")
                        if t % 2 == 0:
                            nc.vector.tensor_scalar_mul(
                                out=ysb, in0=yps,
                                scalar1=float(2.0 ** -(6 + C)),
                            )
                        else:
                            nc.scalar.activation(
                                ysb, yps, AF.Copy,
                                scale=float(2.0 ** -(6 + C)),
                            )
                        nc.sync.dma_start(
                            out=y_out[c * NB + t * P: c * NB + (t + 1) * P, :],
                            in_=ysb,
                        )
    nc.compile()
    return nc


def prepare_inputs(x, Wq, Wkv, Wlin, blin):
    """Host-side quantization/layout. Returns (in_maps, host_const[DM])."""
    import ml_dtypes

    F8NP = ml_dtypes.float8_e4m3
    x = np.asarray(x, dtype=np.float32)
    Wq = np.asarray(Wq, dtype=np.float32)
    Wkv = np.asarray(Wkv, dtype=np.float32)
    Wlin = np.asarray(Wlin, dtype=np.float32)
    blin = np.asarray(blin, dtype=np.float32).reshape(DM)

    b = x.shape[0]
    # centered, transposed x: [B, D, S] fp8
    xt8 = np.ascontiguousarray(
        (2.0 * x - 1.0).transpose(0, 2, 1)).astype(F8NP)
    wq8 = (64.0 * Wq).astype(F8NP)
    # permute Wkv columns: k-cols of all heads first, then v-cols
    wkv3 = Wkv.reshape(D, H, 2 * DH)
    wkv_perm = np.concatenate(
        [wkv3[:, :, :DH].reshape(D, DM), wkv3[:, :, DH:].reshape(D, DM)],
        axis=1,
    )
    wkv8 = (64.0 * wkv_perm).astype(F8NP)
    wlin16 = Wlin.astype(np.float16)

    # exact fp32 consts
    cst_q = 0.5 * Wq.sum(axis=0)                      # [DM]
    cstq_dev = np.ascontiguousarray(
        cst_q.reshape(NJ, P).T).astype(np.float32)    # [P, NJ]
    cst_v = 0.5 * wkv_perm[:, DM:].sum(axis=0)        # [DM] (v-col order = e)
    kappa = SCALE * (cst_v.astype(np.float64)
                     @ wlin16.astype(np.float64))     # [DM]
    host_const = (kappa + blin.astype(np.float64)).astype(np.float32)

    in_maps = [
        {
            "xt": xt8[i],
            "Wq": wq8,
            "Wkv": wkv8,
            "Wlin": wlin16,
            "cstq": cstq_dev,
        }
        for i in range(b)
    ]
    return in_maps, host_const


def finish_output(results, host_const, b):
    """Assemble full y from per-core y16 + host consts."""
    ys = []
    for i in range(b):
        y16 = np.asarray(results[i]["y"]).astype(np.float32)
        ys.append(y16 + host_const[None, :])
    return np.stack(ys)


def kernel(x, Wq, Wkv, Wlin, blin):
    from concourse.bass_utils import run_bass_kernel_spmd

    x = np.asarray(x, dtype=np.float32)
    b = x.shape[0]
    nc = build_nc(x.shape[1])
    in_maps, host_const = prepare_inputs(x, Wq, Wkv, Wlin, blin)
    res = run_bass_kernel_spmd(nc, in_maps, list(range(b)))
    return finish_output(res.results, host_const, b)


if __name__ == "__main__":
    rng = np.random.default_rng(0)
    x = rng.random((B, S, D), dtype=np.float32)
    Wq = (rng.standard_normal((D, DM)) * 0.02).astype(np.float32)
    Wkv = (rng.standard_normal((D, 2 * DM)) * 0.02).astype(np.float32)
    Wlin = (rng.standard_normal((DM, DM)) * 0.02).astype(np.float32)
    blin = np.zeros((DM,), dtype=np.float32)
    y = kernel(x=x, Wq=Wq, Wkv=Wkv, Wlin=Wlin, blin=blin)
    print(y.shape, y.dtype)


# revision 19
# speedup vs baseline: 1.0147x; 1.0147x over previous
"""Trainium2 Bass kernel for nn_Attention (dual-softmax linear attention).

v6: fp8 DoubleRow matmuls for the three large projections (kv-proj, q-proj,
final projection) at ~1.8x the fp16 per-matmul rate, with a centering scheme
that keeps rel_err at fp16 levels (~5e-4 in numpy sim):

  - Host passes x pre-transposed AND centered: xt8 = fp8(2x-1) [D, S].
    No DMA/PE transposes on device at all.
  - Wq8 = fp8(64*Wq), Wkv8 = fp8(64*Wkv) with Wkv columns PERMUTED so all
    k-columns come first (cols 0:1024 = k of heads 0..15), then v-columns.
  - q logits: q = (xt8.T @ Wq8)/128 + cst_q, cst_q = 0.5*colsum(Wq) passed
    from host in fp32 and applied as the per-partition activation bias of
    the Exp evac (exact restoration of the x-mean term).
  - k logits: k~ = (xt8.T @ Wk8)/128 WITHOUT the constant — a per-column
    constant on k cancels in the k-softmax normalization.
  - v: v~ = (xt8.T @ Wv8)/128 without its constant; the v-constant's entire
    contribution to y collapses (softmax weights sum to 1) to a per-output-
    column constant added on the HOST in fp32:
        kappa[c] = SCALE * sum_e 0.5*colsum(Wv)[e] * Wlin16[e, c]
    Removing the dominant rank-1 component of ctx from the device path is
    what makes fp8 quantization of the eq8/W28 phase-B operands harmless.
  - y output in fp16; host adds kappa + blin in fp32.

Engine-load structure (v6):
  - ctx and colsum accumulate in PSUM across ALL chunks (no DVE adds): a
    ones-column appended to each v-tile folds colsum into the ctx matmul
    (out[d, 0:128] = ctx[d, e], out[d, 128] = colsum[d]), one matmul +
    one ldweights per (chunk, t, j). Three packed PSUM banks hold all 8
    head-pair accumulators for the whole of phase A.
  - ctx lands [d, e]; the finalize transposes the two 64x64 diagonal
    blocks to bdt16 [e, d] via DVE 32x32 stream-transposes (PSUM -> SBUF).
  - eq8 = fp8(64*eqn) via a single DVE multiply (rr = 64/rowsum).

Sharding: data-parallel over batch B=8 -> one batch element per NeuronCore.
"""

import numpy as np

import concourse.bass as bass
import concourse.mybir as mybir
from concourse import bacc
from concourse.tile import TileContext
from concourse.masks import make_identity

F32 = mybir.dt.float32
F16 = mybir.dt.float16
F8 = mybir.dt.float8e4
AF = mybir.ActivationFunctionType
DR = mybir.MatmulPerfMode.DoubleRow

S, D = 4096, 1024
H, DH = 16, 64
DM = H * DH  # 1024
B = 8
SCALE = DH ** (-0.5)
C = 17  # W2 fixed-point exponent

P = 128          # partitions
NB = 512         # moving free-dim tile
ND = D // P      # 8 d-tiles
NDP = ND // 2    # 4 d-tile pairs (DoubleRow)
NJ = DM // P     # 8 dout-tiles
CW = 132         # ctx psum region pitch (129 used + pad)


def build_nc(s_len=S):
    sc = s_len // NB
    nc = bacc.Bacc(None, target_bir_lowering=False)

    xt_in = nc.declare_dram_parameter("xt", [D, s_len], F8, isOutput=False)
    wq_in = nc.declare_dram_parameter("Wq", [D, DM], F8, isOutput=False)
    wkv_in = nc.declare_dram_parameter("Wkv", [D, 2 * DM], F8, isOutput=False)
    wlin_in = nc.declare_dram_parameter("Wlin", [DM, DM], F16, isOutput=False)
    cstq_in = nc.declare_dram_parameter("cstq", [P, NJ], F32, isOutput=False)
    y_out = nc.declare_dram_parameter("y", [s_len, DM], F16, isOutput=True)

    with TileContext(nc) as tc:
        from contextlib import ExitStack

        with ExitStack() as stk:
            consts = stk.enter_context(tc.tile_pool(name="consts", bufs=1))
            wbig = stk.enter_context(tc.tile_pool(name="wbig", bufs=1))

            ident = consts.tile([P, P], F16, tag="ident")
            make_identity(nc, ident)
            blkones = consts.tile([P, P], F16, tag="blkones")
            nc.vector.memset(blkones, 0.0)
            nc.vector.memset(blkones[0:64, 0:64], 1.0 / 64)
            nc.vector.memset(blkones[64:128, 64:128], 1.0 / 64)
            zeros396 = consts.tile([P, 3 * CW], F16, tag="zeros396")
            nc.vector.memset(zeros396, 0.0)
            cstq_sb = consts.tile([P, NJ], F32, tag="cstq")
            nc.sync.dma_start(out=cstq_sb, in_=cstq_in[0:P, 0:NJ])

            # blockdiag ctx^T staging tiles (off-diag zeros set once)
            bdt_tiles = []
            for j in range(NJ):
                bdt = consts.tile([P, P], F16, tag=f"bdt{j}", name=f"bdt{j}")
                nc.vector.memset(bdt, 0.0)
                bdt_tiles.append(bdt)

            wkv_sb = wbig.tile([P, ND, 2 * DM], F8, tag="wkv", name="wkv")
            wq_sb = wbig.tile([P, ND, DM], F8, tag="wq", name="wq")
            wlin_sb = wbig.tile([P, ND, DM], F16, tag="wlin", name="wlin")
            w28_sb = wbig.tile([P, NJ, DM], F8, tag="w28", name="w28")

            xt_pool = stk.enter_context(tc.tile_pool(name="xt", bufs=3))
            ek_pool = stk.enter_context(tc.tile_pool(name="ek", bufs=1))
            vt_pool = stk.enter_context(tc.tile_pool(name="vt", bufs=1))
            eq_pool = stk.enter_context(tc.tile_pool(name="eq", bufs=1))
            rr_pool = stk.enter_context(tc.tile_pool(name="rr", bufs=2))
            e8_pool = stk.enter_context(tc.tile_pool(name="e8", bufs=1))
            e8_res = [None] * sc
            rcs_tiles = [None] * NJ

            # v tiles carry a ones column at [:, :, P] so colsum folds into
            # the ctx matmul; set it once per buffer here
            for par in range(2):
                for t in range(4):
                    vt = vt_pool.tile([P, NJ, P + 1], F8,
                                      tag=f"v{par}_{t}", name=f"v{par}_{t}")
                    nc.vector.memset(vt[:, :, P:P + 1], 1.0)

            def load_xt(c, xt_t):
                for jd in range(ND):
                    nc.sync.dma_start(
                        out=xt_t[:, jd, :],
                        in_=xt_in[jd * P:(jd + 1) * P, c * NB:(c + 1) * NB],
                    )

            # ---------------- phase A ----------------
            with (
                tc.tile_pool(name="kvp", bufs=2, space="PSUM") as kvp_pool,
                tc.tile_pool(name="ctxp", bufs=1, space="PSUM") as ctxp_pool,
                tc.tile_pool(name="qp", bufs=2, space="PSUM") as qp_pool,
            ):
                # packed ctx+colsum accumulators: 3 head-pairs per bank,
                # alive across all of phase A
                ctxg = [
                    ctxp_pool.tile([P, 3 * CW], F32, tag=f"ctxg{g}",
                                   name=f"ctxg{g}")
                    for g in range(3)
                ]

                def ctx_region(j):
                    return ctxg[j // 3], (j % 3) * CW

                # initialize each ctx bank with a zero matmul carrying the
                # only start=True (start clears has_written BANK-wide on HW);
                # all real ctx matmuls then accumulate with start=False
                for w in range(18):
                    nc.tensor.matmul(
                        ctxg[w % 3], blkones, zeros396, start=True, stop=True,
                        skip_group_check=True,
                    )

                for c in range(sc):
                    xt_t = xt_pool.tile([P, ND, NB], F8, tag="xt")
                    if c == 0:
                        # just-in-time startup feed: sync carries xt with odd
                        # weight tiles interleaved, gpsimd carries even tiles;
                        # nothing rides the scalar queue (DMAs occupy the
                        # trigger engine's track and would delay evacs)
                        def wkv_dma(eng, jd, half):
                            eng.dma_start(
                                out=wkv_sb[:, jd, half * DM:(half + 1) * DM],
                                in_=wkv_in[jd * P:(jd + 1) * P,
                                           half * DM:(half + 1) * DM],
                            )

                        for jd in range(0, ND, 2):
                            wkv_dma(nc.gpsimd, jd, 0)
                        for jd in range(ND):
                            nc.sync.dma_start(
                                out=xt_t[:, jd, :],
                                in_=xt_in[jd * P:(jd + 1) * P,
                                          c * NB:(c + 1) * NB],
                            )
                            if jd % 2 == 0:
                                wkv_dma(nc.sync, jd + 1, 0)
                        for jd in range(0, ND, 2):
                            wkv_dma(nc.gpsimd, jd, 1)
                        for jd in range(1, ND, 2):
                            wkv_dma(nc.sync, jd, 1)
                        for jd in range(ND):
                            eng = nc.gpsimd if jd % 2 == 0 else nc.sync
                            eng.dma_start(
                                out=wq_sb[:, jd, :],
                                in_=wq_in[jd * P:(jd + 1) * P, :],
                            )
                        for jd in range(ND):
                            eng = nc.gpsimd if jd % 2 == 0 else nc.sync
                            eng.dma_start(
                                out=wlin_sb[:, jd, :],
                                in_=wlin_in[jd * P:(jd + 1) * P, :],
                            )
                    else:
                        load_xt(c, xt_t)

                    # kv projection (fp8 DoubleRow, K=256 per matmul)
                    ek_tiles = [None] * 4
                    v_tiles = [None] * 4
                    for t in range(4):
                        ek_tiles[t] = ek_pool.tile(
                            [P, DM], F8, tag=f"ek{c % 2}_{t}",
                            name=f"ek{c}_{t}"
                        )
                        v_tiles[t] = vt_pool.tile(
                            [P, NJ, P + 1], F8, tag=f"v{c % 2}_{t}",
                            name=f"vt{c}_{t}"
                        )
                    # k-half for all t first (weights stream k-cols first),
                    # then v-half; v evacs ride DVE to offload ScalarE
                    for half in range(2):
                        for n in range(2):
                            for t in range(4):
                                kvps = kvp_pool.tile([P, NB], F32, tag="kvp")
                                for jp in range(NDP):
                                    nc.tensor.matmul(
                                        kvps,
                                        xt_t[:, 2 * jp:2 * jp + 2,
                                             t * P:(t + 1) * P],
                                        wkv_sb[:, 2 * jp:2 * jp + 2,
                                               half * DM + n * NB:
                                               half * DM + (n + 1) * NB],
                                        start=(jp == 0),
                                        stop=(jp == NDP - 1),
                                        perf_mode=DR,
                                    )
                                if half == 0:
                                    nc.scalar.activation(
                                        ek_tiles[t][:, n * NB:(n + 1) * NB],
                                        kvps, AF.Exp, scale=1.0 / 128,
                                    )
                                else:
                                    nc.vector.tensor_scalar_mul(
                                        out=v_tiles[t][:, 4 * n:4 * n + 4, 0:P],
                                        in0=kvps.rearrange(
                                            "p (j e) -> p j e", j=4),
                                        scalar1=1.0 / 128,
                                    )

                    def ctx_block():
                        # ctx+colsum accumulate in PSUM across all chunks
                        for j in range(NJ):
                            cg, base = ctx_region(j)
                            for t in range(4):
                                # start=True clears has_written BANK-wide on
                                # HW, so only the bank's first matmul may
                                # carry it; co-tenant regions overwrite on
                                # virgin has_written instead
                                nc.tensor.matmul(
                                    cg[:, base:base + P + 1],
                                    ek_tiles[t][:, j * P:(j + 1) * P],
                                    v_tiles[t][:, j, 0:P + 1],
                                    start=False,
                                    stop=False,
                                    skip_group_check=True,
                                )

                    if c == sc - 1:
                        ctx_block()
                        # finalize inline, ahead of q/rowsum so their DVE
                        # backlog doesn't delay W28 (phase B's input)
                        bsrc_tiles = [None] * NJ
                        for j in range(NJ):
                            cg, base = ctx_region(j)
                            bsrc = consts.tile(
                                [P, P], F16, tag=f"bsrc{j}", name=f"bsrc{j}"
                            )
                            for db in range(2):
                                o = 64 * db
                                nc.scalar.activation(
                                    bsrc[o:o + 64, o:o + 64],
                                    cg[o:o + 64, base + o:base + o + 64],
                                    AF.Copy,
                                )
                            bsrc_tiles[j] = bsrc
                            cs_sb = consts.tile(
                                [P, 1], F32, tag=f"cs{j}", name=f"cs{j}"
                            )
                            nc.scalar.activation(
                                cs_sb, cg[:, base + P:base + P + 1], AF.Copy
                            )
                            rcs = consts.tile([P, 1], F32, tag=f"rcs{j}")
                            nc.vector.reciprocal_approx_fast(
                                out=rcs, in_=cs_sb
                            )
                            rcs_tiles[j] = rcs
                        for j in range(NJ):
                            trps = qp_pool.tile([P, NB], F32, tag="qp")
                            nc.tensor.matmul(
                                trps[:, 0:P], bsrc_tiles[j], ident
                            )
                            for db in range(2):
                                o = 64 * db
                                nc.scalar.activation(
                                    bdt_tiles[j][o:o + 64, o:o + 64],
                                    trps[o:o + 64, o:o + 64],
                                    AF.Copy,
                                )
                        for j in range(NJ):
                            for n in range(2):
                                w2ps = qp_pool.tile([P, NB], F32, tag="qp")
                                nc.tensor.matmul(
                                    w2ps,
                                    bdt_tiles[j],
                                    wlin_sb[:, j, n * NB:(n + 1) * NB],
                                )
                                nc.vector.tensor_scalar(
                                    out=w28_sb[:, j, n * NB:(n + 1) * NB],
                                    in0=w2ps,
                                    scalar1=rcs_tiles[j],
                                    scalar2=SCALE * float(2.0 ** C),
                                    op0=mybir.AluOpType.mult,
                                    op1=mybir.AluOpType.mult,
                                )

                    # q projection (fp8 DoubleRow) -> eq16
                    e8_t = e8_pool.tile([P, NJ, NB], F8, tag=f"e8_{c}")
                    e8_res[c] = e8_t
                    eq16_tiles = [None] * NJ
                    for j in range(NJ):
                        qps = qp_pool.tile([P, NB], F32, tag="qp")
                        for jp in range(NDP):
                            nc.tensor.matmul(
                                qps,
                                wq_sb[:, 2 * jp:2 * jp + 2, j * P:(j + 1) * P],
                                xt_t[:, 2 * jp:2 * jp + 2, :],
                                start=(jp == 0),
                                stop=(jp == NDP - 1),
                                perf_mode=DR,
                            )
                        eq16 = eq_pool.tile([P, NB], F16, tag=f"eq{j}")
                        nc.scalar.activation(
                            eq16, qps, AF.Exp,
                            scale=1.0 / 128, bias=cstq_sb[:, j:j + 1],
                        )
                        eq16_tiles[j] = eq16

                    # rowsum (1/64-blockones matmul) -> rr -> eq8 = eq16*rr
                    for j in range(NJ):
                        eq16 = eq16_tiles[j]
                        rsps = qp_pool.tile([P, NB], F32, tag="qp")
                        nc.tensor.matmul(rsps, blkones, eq16)
                        rr = rr_pool.tile([P, NB], F32, tag="rr")
                        nc.vector.reciprocal_approx_fast(out=rr, in_=rsps)
                        nc.vector.tensor_mul(e8_t[:, j, :], eq16, rr)

                    if c < sc - 1:
                        ctx_block()

            y_pool = stk.enter_context(tc.tile_pool(name="ysb", bufs=3))

            # ---------------- phase B: y = eq8.T @ W28 (fp8 DoubleRow)
            with tc.tile_pool(name="yp", bufs=4, space="PSUM") as yp_pool:
                for c in range(sc):
                    for t in range(4):
                        yps = yp_pool.tile([P, DM], F32, tag="yp")
                        for n in range(2):
                            for jp in range(NJ // 2):
                                nc.tensor.matmul(
                                    yps[:, n * NB:(n + 1) * NB],
                                    e8_res[c][:, 2 * jp:2 * jp + 2,
                                              t * P:(t + 1) * P],
                                    w28_sb[:, 2 * jp:2 * jp + 2,
                                           n * NB:(n + 1) * NB],
                                    start=(jp == 0),
                                    stop=(jp == NJ // 2 - 1),
                                    perf_mode=DR,
                                )
                        ysb = y_pool.tile([P, DM], F16, tag="ysb")
                        if t % 2 == 0:
                            nc.vector.tensor_scalar_mul(
                                out=ysb, in0=yps,
                                scalar1=float(2.0 ** -(6 + C)),
                            )
                        else:
                            nc.scalar.activation(
                                ysb, yps, AF.Copy,
                                scale=float(2.0 ** -(6 + C)),
                            )
                        nc.sync.dma_start(
                            out=y_out[c * NB + t * P: c * NB + (t + 1) * P, :],
                            in_=ysb,
                        )
    nc.compile()
    return nc


def prepare_inputs(x, Wq, Wkv, Wlin, blin):
    """Host-side quantization/layout. Returns (in_maps, host_const[DM])."""
    import ml_dtypes

    F8NP = ml_dtypes.float8_e4m3
    x = np.asarray(x, dtype=np.float32)
    Wq = np.asarray(Wq, dtype=np.float32)
    Wkv = np.asarray(Wkv, dtype=np.float32)
    Wlin = np.asarray(Wlin, dtype=np.float32)
    blin = np.asarray(blin, dtype=np.float32).reshape(DM)

    b = x.shape[0]
    # centered, transposed x: [B, D, S] fp8
    xt8 = np.ascontiguousarray(
        (2.0 * x - 1.0).transpose(0, 2, 1)).astype(F8NP)
    wq8 = (64.0 * Wq).astype(F8NP)
    # permute Wkv columns: k-cols of all heads first, then v-cols
    wkv3 = Wkv.reshape(D, H, 2 * DH)
    wkv_perm = np.concatenate(
        [wkv3[:, :, :DH].reshape(D, DM), wkv3[:, :, DH:].reshape(D, DM)],
        axis=1,
    )
    wkv8 = (64.0 * wkv_perm).astype(F8NP)
    wlin16 = Wlin.astype(np.float16)

    # exact fp32 consts
    cst_q = 0.5 * Wq.sum(axis=0)                      # [DM]
    cstq_dev = np.ascontiguousarray(
        cst_q.reshape(NJ, P).T).astype(np.float32)    # [P, NJ]
    cst_v = 0.5 * wkv_perm[:, DM:].sum(axis=0)        # [DM] (v-col order = e)
    kappa = SCALE * (cst_v.astype(np.float64)
                     @ wlin16.astype(np.float64))     # [DM]
    host_const = (kappa + blin.astype(np.float64)).astype(np.float32)

    in_maps = [
        {
            "xt": xt8[i],
            "Wq": wq8,
            "Wkv": wkv8,
            "Wlin": wlin16,
            "cstq": cstq_dev,
        }
        for i in range(b)
    ]
    return in_maps, host_const


def finish_output(results, host_const, b):
    """Assemble full y from per-core y16 + host consts."""
    ys = []
    for i in range(b):
        y16 = np.asarray(results[i]["y"]).astype(np.float32)
        ys.append(y16 + host_const[None, :])
    return np.stack(ys)


def kernel(x, Wq, Wkv, Wlin, blin):
    from concourse.bass_utils import run_bass_kernel_spmd

    x = np.asarray(x, dtype=np.float32)
    b = x.shape[0]
    nc = build_nc(x.shape[1])
    in_maps, host_const = prepare_inputs(x, Wq, Wkv, Wlin, blin)
    res = run_bass_kernel_spmd(nc, in_maps, list(range(b)))
    return finish_output(res.results, host_const, b)


if __name__ == "__main__":
    rng = np.random.default_rng(0)
    x = rng.random((B, S, D), dtype=np.float32)
    Wq = (rng.standard_normal((D, DM)) * 0.02).astype(np.float32)
    Wkv = (rng.standard_normal((D, 2 * DM)) * 0.02).astype(np.float32)
    Wlin = (rng.standard_normal((DM, DM)) * 0.02).astype(np.float32)
    blin = np.zeros((DM,), dtype=np.float32)
    y = kernel(x=x, Wq=Wq, Wkv=Wkv, Wlin=Wlin, blin=blin)
    print(y.shape, y.dtype)


# revision 20
# speedup vs baseline: 1.0395x; 1.0245x over previous
"""Trainium2 Bass kernel for nn_Attention (dual-softmax linear attention).

v6: fp8 DoubleRow matmuls for the three large projections (kv-proj, q-proj,
final projection) at ~1.8x the fp16 per-matmul rate, with a centering scheme
that keeps rel_err at fp16 levels (~5e-4 in numpy sim):

  - Host passes x pre-transposed AND centered: xt8 = fp8(2x-1) [D, S].
    No DMA/PE transposes on device at all.
  - Wq8 = fp8(64*Wq), Wkv8 = fp8(64*Wkv) with Wkv columns PERMUTED so all
    k-columns come first (cols 0:1024 = k of heads 0..15), then v-columns.
  - q logits: q = (xt8.T @ Wq8)/128 + cst_q, cst_q = 0.5*colsum(Wq) passed
    from host in fp32 and applied as the per-partition activation bias of
    the Exp evac (exact restoration of the x-mean term).
  - k logits: k~ = (xt8.T @ Wk8)/128 WITHOUT the constant — a per-column
    constant on k cancels in the k-softmax normalization.
  - v: v~ = (xt8.T @ Wv8)/128 without its constant; the v-constant's entire
    contribution to y collapses (softmax weights sum to 1) to a per-output-
    column constant added on the HOST in fp32:
        kappa[c] = SCALE * sum_e 0.5*colsum(Wv)[e] * Wlin16[e, c]
    Removing the dominant rank-1 component of ctx from the device path is
    what makes fp8 quantization of the eq8/W28 phase-B operands harmless.
  - y output in fp16; host adds kappa + blin in fp32.

Engine-load structure (v6):
  - ctx and colsum accumulate in PSUM across ALL chunks (no DVE adds): a
    ones-column appended to each v-tile folds colsum into the ctx matmul
    (out[d, 0:128] = ctx[d, e], out[d, 128] = colsum[d]), one matmul +
    one ldweights per (chunk, t, j). Three packed PSUM banks hold all 8
    head-pair accumulators for the whole of phase A.
  - ctx lands [d, e]; the finalize transposes the two 64x64 diagonal
    blocks to bdt16 [e, d] via DVE 32x32 stream-transposes (PSUM -> SBUF).
  - eq8 = fp8(64*eqn) via a single DVE multiply (rr = 64/rowsum).

Sharding: data-parallel over batch B=8 -> one batch element per NeuronCore.
"""

import numpy as np

import concourse.bass as bass
import concourse.mybir as mybir
from concourse import bacc
from concourse.tile import TileContext
from concourse.masks import make_identity

F32 = mybir.dt.float32
F16 = mybir.dt.float16
F8 = mybir.dt.float8e4
AF = mybir.ActivationFunctionType
DR = mybir.MatmulPerfMode.DoubleRow

S, D = 4096, 1024
H, DH = 16, 64
DM = H * DH  # 1024
B = 8
SCALE = DH ** (-0.5)
C = 17  # W2 fixed-point exponent

P = 128          # partitions
NB = 512         # moving free-dim tile
ND = D // P      # 8 d-tiles
NDP = ND // 2    # 4 d-tile pairs (DoubleRow)
NJ = DM // P     # 8 dout-tiles
CW = 132         # ctx psum region pitch (129 used + pad)


def build_nc(s_len=S):
    sc = s_len // NB
    nc = bacc.Bacc(None, target_bir_lowering=False)

    xt_in = nc.declare_dram_parameter("xt", [D, s_len], F8, isOutput=False)
    wq_in = nc.declare_dram_parameter("Wq", [D, DM], F8, isOutput=False)
    wkv_in = nc.declare_dram_parameter("Wkv", [D, 2 * DM], F8, isOutput=False)
    wlin_in = nc.declare_dram_parameter("Wlin", [DM, DM], F16, isOutput=False)
    cstq_in = nc.declare_dram_parameter("cstq", [P, NJ], F32, isOutput=False)
    y_out = nc.declare_dram_parameter("y", [s_len, DM], F16, isOutput=True)

    with TileContext(nc) as tc:
        from contextlib import ExitStack

        with ExitStack() as stk:
            consts = stk.enter_context(tc.tile_pool(name="consts", bufs=1))
            wbig = stk.enter_context(tc.tile_pool(name="wbig", bufs=1))

            ident = consts.tile([P, P], F16, tag="ident")
            make_identity(nc, ident)
            blkones = consts.tile([P, P], F16, tag="blkones")
            nc.vector.memset(blkones, 0.0)
            nc.vector.memset(blkones[0:64, 0:64], 1.0 / 64)
            nc.vector.memset(blkones[64:128, 64:128], 1.0 / 64)
            zeros396 = consts.tile([P, 3 * CW], F16, tag="zeros396")
            nc.vector.memset(zeros396, 0.0)
            cstq_sb = consts.tile([P, NJ], F32, tag="cstq")
            nc.sync.dma_start(out=cstq_sb, in_=cstq_in[0:P, 0:NJ])

            # blockdiag ctx^T staging tiles (off-diag zeros set once)
            bdt_tiles = []
            for j in range(NJ):
                bdt = consts.tile([P, P], F16, tag=f"bdt{j}", name=f"bdt{j}")
                nc.vector.memset(bdt, 0.0)
                bdt_tiles.append(bdt)

            wkv_sb = wbig.tile([P, ND, 2 * DM], F8, tag="wkv", name="wkv")
            wq_sb = wbig.tile([P, ND, DM], F8, tag="wq", name="wq")
            wlin_sb = wbig.tile([P, ND, DM], F16, tag="wlin", name="wlin")
            w28_sb = wbig.tile([P, NJ, DM], F8, tag="w28", name="w28")

            xt_pool = stk.enter_context(tc.tile_pool(name="xt", bufs=3))
            ek_pool = stk.enter_context(tc.tile_pool(name="ek", bufs=1))
            vt_pool = stk.enter_context(tc.tile_pool(name="vt", bufs=1))
            eq_pool = stk.enter_context(tc.tile_pool(name="eq", bufs=1))
            rr_pool = stk.enter_context(tc.tile_pool(name="rr", bufs=1))
            e8_pool = stk.enter_context(tc.tile_pool(name="e8", bufs=1))
            e8_res = [None] * sc
            rcs_tiles = [None] * NJ

            # v tiles carry a ones column at [:, :, P] so colsum folds into
            # the ctx matmul; set it once per buffer here
            for par in range(2):
                for t in range(4):
                    vt = vt_pool.tile([P, NJ, P + 1], F8,
                                      tag=f"v{par}_{t}", name=f"v{par}_{t}")
                    nc.vector.memset(vt[:, :, P:P + 1], 1.0)

            def load_xt(c, xt_t):
                for jd in range(ND):
                    nc.sync.dma_start(
                        out=xt_t[:, jd, :],
                        in_=xt_in[jd * P:(jd + 1) * P, c * NB:(c + 1) * NB],
                    )

            # ---------------- phase A ----------------
            with (
                tc.tile_pool(name="kvp", bufs=2, space="PSUM") as kvp_pool,
                tc.tile_pool(name="ctxp", bufs=1, space="PSUM") as ctxp_pool,
                tc.tile_pool(name="qp", bufs=2, space="PSUM") as qp_pool,
                tc.tile_pool(name="rsp7", bufs=1, space="PSUM") as rsp7_pool,
            ):
                # packed ctx+colsum accumulators: 3 head-pairs per bank,
                # alive across all of phase A
                ctxg = [
                    ctxp_pool.tile([P, 3 * CW], F32, tag=f"ctxg{g}",
                                   name=f"ctxg{g}")
                    for g in range(3)
                ]

                def ctx_region(j):
                    return ctxg[j // 3], (j % 3) * CW

                # initialize each ctx bank with a zero matmul carrying the
                # only start=True (start clears has_written BANK-wide on HW);
                # all real ctx matmuls then accumulate with start=False
                for w in range(18):
                    nc.tensor.matmul(
                        ctxg[w % 3], blkones, zeros396, start=True, stop=True,
                        skip_group_check=True,
                    )

                for c in range(sc):
                    xt_t = xt_pool.tile([P, ND, NB], F8, tag="xt")
                    if c == 0:
                        # just-in-time startup feed: sync carries xt with odd
                        # weight tiles interleaved, gpsimd carries even tiles;
                        # nothing rides the scalar queue (DMAs occupy the
                        # trigger engine's track and would delay evacs)
                        def wkv_dma(eng, jd, half):
                            eng.dma_start(
                                out=wkv_sb[:, jd, half * DM:(half + 1) * DM],
                                in_=wkv_in[jd * P:(jd + 1) * P,
                                           half * DM:(half + 1) * DM],
                            )

                        for jd in range(0, ND, 2):
                            wkv_dma(nc.gpsimd, jd, 0)
                        for jd in range(ND):
                            nc.sync.dma_start(
                                out=xt_t[:, jd, :],
                                in_=xt_in[jd * P:(jd + 1) * P,
                                          c * NB:(c + 1) * NB],
                            )
                            if jd % 2 == 0:
                                wkv_dma(nc.sync, jd + 1, 0)
                        for jd in range(0, ND, 2):
                            wkv_dma(nc.gpsimd, jd, 1)
                        for jd in range(1, ND, 2):
                            wkv_dma(nc.sync, jd, 1)
                        for jd in range(ND):
                            eng = nc.gpsimd if jd % 2 == 0 else nc.sync
                            eng.dma_start(
                                out=wq_sb[:, jd, :],
                                in_=wq_in[jd * P:(jd + 1) * P, :],
                            )
                        for jd in range(ND):
                            eng = nc.gpsimd if jd % 2 == 0 else nc.sync
                            eng.dma_start(
                                out=wlin_sb[:, jd, :],
                                in_=wlin_in[jd * P:(jd + 1) * P, :],
                            )
                    else:
                        load_xt(c, xt_t)

                    # kv projection (fp8 DoubleRow, K=256 per matmul)
                    ek_tiles = [None] * 4
                    v_tiles = [None] * 4
                    for t in range(4):
                        ek_tiles[t] = ek_pool.tile(
                            [P, DM], F8, tag=f"ek{c % 2}_{t}",
                            name=f"ek{c}_{t}"
                        )
                        v_tiles[t] = vt_pool.tile(
                            [P, NJ, P + 1], F8, tag=f"v{c % 2}_{t}",
                            name=f"vt{c}_{t}"
                        )
                    # k-half for all t first (weights stream k-cols first),
                    # then v-half; v evacs ride DVE to offload ScalarE
                    for half in range(2):
                        for n in range(2):
                            for t in range(4):
                                kvps = kvp_pool.tile([P, NB], F32, tag="kvp")
                                for jp in range(NDP):
                                    nc.tensor.matmul(
                                        kvps,
                                        xt_t[:, 2 * jp:2 * jp + 2,
                                             t * P:(t + 1) * P],
                                        wkv_sb[:, 2 * jp:2 * jp + 2,
                                               half * DM + n * NB:
                                               half * DM + (n + 1) * NB],
                                        start=(jp == 0),
                                        stop=(jp == NDP - 1),
                                        perf_mode=DR,
                                    )
                                if half == 0:
                                    nc.scalar.activation(
                                        ek_tiles[t][:, n * NB:(n + 1) * NB],
                                        kvps, AF.Exp, scale=1.0 / 128,
                                    )
                                else:
                                    nc.vector.tensor_scalar_mul(
                                        out=v_tiles[t][:, 4 * n:4 * n + 4, 0:P],
                                        in0=kvps.rearrange(
                                            "p (j e) -> p j e", j=4),
                                        scalar1=1.0 / 128,
                                    )

                    def ctx_block():
                        # ctx+colsum accumulate in PSUM across all chunks
                        for j in range(NJ):
                            cg, base = ctx_region(j)
                            for t in range(4):
                                # start=True clears has_written BANK-wide on
                                # HW, so only the bank's first matmul may
                                # carry it; co-tenant regions overwrite on
                                # virgin has_written instead
                                nc.tensor.matmul(
                                    cg[:, base:base + P + 1],
                                    ek_tiles[t][:, j * P:(j + 1) * P],
                                    v_tiles[t][:, j, 0:P + 1],
                                    start=False,
                                    stop=False,
                                    skip_group_check=True,
                                )

                    if c == sc - 1:
                        ctx_block()
                        # finalize inline, ahead of q/rowsum so their DVE
                        # backlog doesn't delay W28 (phase B's input)
                        bsrc_tiles = [None] * NJ
                        for j in range(NJ):
                            cg, base = ctx_region(j)
                            bsrc = consts.tile(
                                [P, P], F16, tag=f"bsrc{j}", name=f"bsrc{j}"
                            )
                            for db in range(2):
                                o = 64 * db
                                nc.scalar.activation(
                                    bsrc[o:o + 64, o:o + 64],
                                    cg[o:o + 64, base + o:base + o + 64],
                                    AF.Copy,
                                )
                            bsrc_tiles[j] = bsrc
                            cs_sb = consts.tile(
                                [P, 1], F32, tag=f"cs{j}", name=f"cs{j}"
                            )
                            nc.scalar.activation(
                                cs_sb, cg[:, base + P:base + P + 1], AF.Copy
                            )
                            rcs = consts.tile([P, 1], F32, tag=f"rcs{j}")
                            nc.vector.reciprocal_approx_fast(
                                out=rcs, in_=cs_sb
                            )
                            rcs_tiles[j] = rcs
                        for j in range(NJ):
                            trps = qp_pool.tile([P, NB], F32, tag="qp")
                            nc.tensor.matmul(
                                trps[:, 0:P], bsrc_tiles[j], ident
                            )
                            for db in range(2):
                                o = 64 * db
                                nc.scalar.activation(
                                    bdt_tiles[j][o:o + 64, o:o + 64],
                                    trps[o:o + 64, o:o + 64],
                                    AF.Copy,
                                )
                        for j in range(NJ):
                            for n in range(2):
                                w2ps = qp_pool.tile([P, NB], F32, tag="qp")
                                nc.tensor.matmul(
                                    w2ps,
                                    bdt_tiles[j],
                                    wlin_sb[:, j, n * NB:(n + 1) * NB],
                                )
                                nc.vector.tensor_scalar(
                                    out=w28_sb[:, j, n * NB:(n + 1) * NB],
                                    in0=w2ps,
                                    scalar1=rcs_tiles[j],
                                    scalar2=SCALE * float(2.0 ** C),
                                    op0=mybir.AluOpType.mult,
                                    op1=mybir.AluOpType.mult,
                                )

                    # q projection (fp8 DoubleRow) -> eq16
                    e8_t = e8_pool.tile([P, NJ, NB], F8, tag=f"e8_{c}")
                    e8_res[c] = e8_t
                    eq16_tiles = [None] * NJ
                    for j in range(NJ):
                        qps = qp_pool.tile([P, NB], F32, tag="qp")
                        for jp in range(NDP):
                            nc.tensor.matmul(
                                qps,
                                wq_sb[:, 2 * jp:2 * jp + 2, j * P:(j + 1) * P],
                                xt_t[:, 2 * jp:2 * jp + 2, :],
                                start=(jp == 0),
                                stop=(jp == NDP - 1),
                                perf_mode=DR,
                            )
                        eq16 = eq_pool.tile([P, NB], F16, tag=f"eq{j}")
                        nc.scalar.activation(
                            eq16, qps, AF.Exp,
                            scale=1.0 / 128, bias=cstq_sb[:, j:j + 1],
                        )
                        eq16_tiles[j] = eq16

                    # rowsum (1/64-blockones matmul) -> rr -> eq8 = eq16*rr
                    # all recips issue before the muls so PSUM frees early
                    # (phase B's yp banks wait on the last rsps reader);
                    # chunk sc-1 parks rsps in the spare 8th bank
                    rr_tiles = [None] * NJ
                    for j in range(NJ):
                        pool = rsp7_pool if c == sc - 1 else qp_pool
                        rsps = pool.tile(
                            [P, NB], F32,
                            tag="rsp7" if c == sc - 1 else "qp",
                            name=f"rsps{c}_{j}",
                        )
                        nc.tensor.matmul(rsps, blkones, eq16_tiles[j])
                        rr = rr_pool.tile([P, NB], F32, tag=f"rr{j}",
                                          name=f"rr{c}_{j}")
                        nc.vector.reciprocal_approx_fast(out=rr, in_=rsps)
                        rr_tiles[j] = rr
                    for j in range(NJ):
                        nc.vector.tensor_mul(
                            e8_t[:, j, :], eq16_tiles[j], rr_tiles[j]
                        )

                    if c < sc - 1:
                        ctx_block()

            y_pool = stk.enter_context(tc.tile_pool(name="ysb", bufs=3))

            # ---------------- phase B: y = eq8.T @ W28 (fp8 DoubleRow)
            with tc.tile_pool(name="yp", bufs=3, space="PSUM") as yp_pool:
                for c in range(sc):
                    for t in range(4):
                        yps = yp_pool.tile([P, DM], F32, tag="yp")
                        for n in range(2):
                            for jp in range(NJ // 2):
                                nc.tensor.matmul(
                                    yps[:, n * NB:(n + 1) * NB],
                                    e8_res[c][:, 2 * jp:2 * jp + 2,
                                              t * P:(t + 1) * P],
                                    w28_sb[:, 2 * jp:2 * jp + 2,
                                           n * NB:(n + 1) * NB],
                                    start=(jp == 0),
                                    stop=(jp == NJ // 2 - 1),
                                    perf_mode=DR,
                                )
                        ysb = y_pool.tile([P, DM], F16, tag="ysb")
                        if t % 2 == 0:
                            nc.vector.tensor_scalar_mul(
                                out=ysb, in0=yps,
                                scalar1=float(2.0 ** -(6 + C)),
                            )
                        else:
                            nc.scalar.activation(
                                ysb, yps, AF.Copy,
                                scale=float(2.0 ** -(6 + C)),
                            )
                        nc.sync.dma_start(
                            out=y_out[c * NB + t * P: c * NB + (t + 1) * P, :],
                            in_=ysb,
                        )
    nc.compile()
    return nc


def prepare_inputs(x, Wq, Wkv, Wlin, blin):
    """Host-side quantization/layout. Returns (in_maps, host_const[DM])."""
    import ml_dtypes

    F8NP = ml_dtypes.float8_e4m3
    x = np.asarray(x, dtype=np.float32)
    Wq = np.asarray(Wq, dtype=np.float32)
    Wkv = np.asarray(Wkv, dtype=np.float32)
    Wlin = np.asarray(Wlin, dtype=np.float32)
    blin = np.asarray(blin, dtype=np.float32).reshape(DM)

    b = x.shape[0]
    # centered, transposed x: [B, D, S] fp8
    xt8 = np.ascontiguousarray(
        (2.0 * x - 1.0).transpose(0, 2, 1)).astype(F8NP)
    wq8 = (64.0 * Wq).astype(F8NP)
    # permute Wkv columns: k-cols of all heads first, then v-cols
    wkv3 = Wkv.reshape(D, H, 2 * DH)
    wkv_perm = np.concatenate(
        [wkv3[:, :, :DH].reshape(D, DM), wkv3[:, :, DH:].reshape(D, DM)],
        axis=1,
    )
    wkv8 = (64.0 * wkv_perm).astype(F8NP)
    wlin16 = Wlin.astype(np.float16)

    # exact fp32 consts
    cst_q = 0.5 * Wq.sum(axis=0)                      # [DM]
    cstq_dev = np.ascontiguousarray(
        cst_q.reshape(NJ, P).T).astype(np.float32)    # [P, NJ]
    cst_v = 0.5 * wkv_perm[:, DM:].sum(axis=0)        # [DM] (v-col order = e)
    kappa = SCALE * (cst_v.astype(np.float64)
                     @ wlin16.astype(np.float64))     # [DM]
    host_const = (kappa + blin.astype(np.float64)).astype(np.float32)

    in_maps = [
        {
            "xt": xt8[i],
            "Wq": wq8,
            "Wkv": wkv8,
            "Wlin": wlin16,
            "cstq": cstq_dev,
        }
        for i in range(b)
    ]
    return in_maps, host_const


def finish_output(results, host_const, b):
    """Assemble full y from per-core y16 + host consts."""
    ys = []
    for i in range(b):
        y16 = np.asarray(results[i]["y"]).astype(np.float32)
        ys.append(y16 + host_const[None, :])
    return np.stack(ys)


def kernel(x, Wq, Wkv, Wlin, blin):
    from concourse.bass_utils import run_bass_kernel_spmd

    x = np.asarray(x, dtype=np.float32)
    b = x.shape[0]
    nc = build_nc(x.shape[1])
    in_maps, host_const = prepare_inputs(x, Wq, Wkv, Wlin, blin)
    res = run_bass_kernel_spmd(nc, in_maps, list(range(b)))
    return finish_output(res.results, host_const, b)


if __name__ == "__main__":
    rng = np.random.default_rng(0)
    x = rng.random((B, S, D), dtype=np.float32)
    Wq = (rng.standard_normal((D, DM)) * 0.02).astype(np.float32)
    Wkv = (rng.standard_normal((D, 2 * DM)) * 0.02).astype(np.float32)
    Wlin = (rng.standard_normal((DM, DM)) * 0.02).astype(np.float32)
    blin = np.zeros((DM,), dtype=np.float32)
    y = kernel(x=x, Wq=Wq, Wkv=Wkv, Wlin=Wlin, blin=blin)
    print(y.shape, y.dtype)


# revision 21
# speedup vs baseline: 1.0499x; 1.0100x over previous
"""Trainium2 Bass kernel for nn_Attention (dual-softmax linear attention).

v6: fp8 DoubleRow matmuls for the three large projections (kv-proj, q-proj,
final projection) at ~1.8x the fp16 per-matmul rate, with a centering scheme
that keeps rel_err at fp16 levels (~5e-4 in numpy sim):

  - Host passes x pre-transposed AND centered: xt8 = fp8(2x-1) [D, S].
    No DMA/PE transposes on device at all.
  - Wq8 = fp8(64*Wq), Wkv8 = fp8(64*Wkv) with Wkv columns PERMUTED so all
    k-columns come first (cols 0:1024 = k of heads 0..15), then v-columns.
  - q logits: q = (xt8.T @ Wq8)/128 + cst_q, cst_q = 0.5*colsum(Wq) passed
    from host in fp32 and applied as the per-partition activation bias of
    the Exp evac (exact restoration of the x-mean term).
  - k logits: k~ = (xt8.T @ Wk8)/128 WITHOUT the constant — a per-column
    constant on k cancels in the k-softmax normalization.
  - v: v~ = (xt8.T @ Wv8)/128 without its constant; the v-constant's entire
    contribution to y collapses (softmax weights sum to 1) to a per-output-
    column constant added on the HOST in fp32:
        kappa[c] = SCALE * sum_e 0.5*colsum(Wv)[e] * Wlin16[e, c]
    Removing the dominant rank-1 component of ctx from the device path is
    what makes fp8 quantization of the eq8/W28 phase-B operands harmless.
  - y output in fp16; host adds kappa + blin in fp32.

Engine-load structure (v6):
  - ctx and colsum accumulate in PSUM across ALL chunks (no DVE adds): a
    ones-column appended to each v-tile folds colsum into the ctx matmul
    (out[d, 0:128] = ctx[d, e], out[d, 128] = colsum[d]), one matmul +
    one ldweights per (chunk, t, j). Three packed PSUM banks hold all 8
    head-pair accumulators for the whole of phase A.
  - ctx lands [d, e]; the finalize transposes the two 64x64 diagonal
    blocks to bdt16 [e, d] via DVE 32x32 stream-transposes (PSUM -> SBUF).
  - eq8 = fp8(64*eqn) via a single DVE multiply (rr = 64/rowsum).

Sharding: data-parallel over batch B=8 -> one batch element per NeuronCore.
"""

import numpy as np

import concourse.bass as bass
import concourse.mybir as mybir
from concourse import bacc
from concourse.tile import TileContext
from concourse.masks import make_identity

F32 = mybir.dt.float32
F16 = mybir.dt.float16
F8 = mybir.dt.float8e4
AF = mybir.ActivationFunctionType
DR = mybir.MatmulPerfMode.DoubleRow

S, D = 4096, 1024
H, DH = 16, 64
DM = H * DH  # 1024
B = 8
SCALE = DH ** (-0.5)
C = 17  # W2 fixed-point exponent

P = 128          # partitions
NB = 512         # moving free-dim tile
ND = D // P      # 8 d-tiles
NDP = ND // 2    # 4 d-tile pairs (DoubleRow)
NJ = DM // P     # 8 dout-tiles
CW = 132         # ctx psum region pitch (129 used + pad)


def build_nc(s_len=S):
    sc = s_len // NB
    nc = bacc.Bacc(None, target_bir_lowering=False)

    xt_in = nc.declare_dram_parameter("xt", [D, s_len], F8, isOutput=False)
    wq_in = nc.declare_dram_parameter("Wq", [D, DM], F8, isOutput=False)
    wkv_in = nc.declare_dram_parameter("Wkv", [D, 2 * DM], F8, isOutput=False)
    wlin_in = nc.declare_dram_parameter("Wlin", [DM, DM], F16, isOutput=False)
    cstq_in = nc.declare_dram_parameter("cstq", [P, NJ], F32, isOutput=False)
    y_out = nc.declare_dram_parameter("y", [s_len, DM], F16, isOutput=True)

    with TileContext(nc) as tc:
        from contextlib import ExitStack

        with ExitStack() as stk:
            consts = stk.enter_context(tc.tile_pool(name="consts", bufs=1))
            wbig = stk.enter_context(tc.tile_pool(name="wbig", bufs=1))

            ident = consts.tile([P, P], F16, tag="ident")
            make_identity(nc, ident)
            blkones = consts.tile([P, P], F16, tag="blkones")
            nc.vector.memset(blkones, 0.0)
            nc.vector.memset(blkones[0:64, 0:64], 1.0 / 64)
            nc.vector.memset(blkones[64:128, 64:128], 1.0 / 64)
            zeros396 = consts.tile([P, 3 * CW], F16, tag="zeros396")
            nc.vector.memset(zeros396, 0.0)
            cstq_sb = consts.tile([P, NJ], F32, tag="cstq")
            nc.sync.dma_start(out=cstq_sb, in_=cstq_in[0:P, 0:NJ])

            # blockdiag ctx^T staging tiles (off-diag zeros set once)
            bdt_tiles = []
            for j in range(NJ):
                bdt = consts.tile([P, P], F16, tag=f"bdt{j}", name=f"bdt{j}")
                nc.vector.memset(bdt, 0.0)
                bdt_tiles.append(bdt)

            wkv_sb = wbig.tile([P, ND, 2 * DM], F8, tag="wkv", name="wkv")
            wq_sb = wbig.tile([P, ND, DM], F8, tag="wq", name="wq")
            wlin_sb = wbig.tile([P, ND, DM], F16, tag="wlin", name="wlin")
            w28_sb = wbig.tile([P, NJ, DM], F8, tag="w28", name="w28")

            xt_pool = stk.enter_context(tc.tile_pool(name="xt", bufs=3))
            ek_pool = stk.enter_context(tc.tile_pool(name="ek", bufs=1))
            vt_pool = stk.enter_context(tc.tile_pool(name="vt", bufs=1))
            eq_pool = stk.enter_context(tc.tile_pool(name="eq", bufs=1))
            rr_pool = stk.enter_context(tc.tile_pool(name="rr", bufs=1))
            e8_pool = stk.enter_context(tc.tile_pool(name="e8", bufs=1))
            e8_res = [None] * sc
            rcs_tiles = [None] * NJ

            # v tiles carry a ones column at [:, :, P] so colsum folds into
            # the ctx matmul; set it once per buffer here
            for par in range(2):
                for t in range(4):
                    vt = vt_pool.tile([P, NJ, P + 1], F8,
                                      tag=f"v{par}_{t}", name=f"v{par}_{t}")
                    nc.vector.memset(vt[:, :, P:P + 1], 1.0)

            def load_xt(c, xt_t):
                for jd in range(ND):
                    nc.sync.dma_start(
                        out=xt_t[:, jd, :],
                        in_=xt_in[jd * P:(jd + 1) * P, c * NB:(c + 1) * NB],
                    )

            # ---------------- phase A ----------------
            with (
                tc.tile_pool(name="kvp", bufs=2, space="PSUM") as kvp_pool,
                tc.tile_pool(name="ctxp", bufs=1, space="PSUM") as ctxp_pool,
                tc.tile_pool(name="qp", bufs=2, space="PSUM") as qp_pool,
                tc.tile_pool(name="rsp7", bufs=1, space="PSUM") as rsp7_pool,
            ):
                # packed ctx+colsum accumulators: 3 head-pairs per bank,
                # alive across all of phase A
                ctxg = [
                    ctxp_pool.tile([P, 3 * CW], F32, tag=f"ctxg{g}",
                                   name=f"ctxg{g}")
                    for g in range(3)
                ]

                def ctx_region(j):
                    return ctxg[j // 3], (j % 3) * CW

                # initialize each ctx bank with a zero matmul carrying the
                # only start=True (start clears has_written BANK-wide on HW);
                # all real ctx matmuls then accumulate with start=False
                for w in range(28):
                    nc.tensor.matmul(
                        ctxg[w % 3], blkones, zeros396, start=True, stop=True,
                        skip_group_check=True,
                    )

                for c in range(sc):
                    xt_t = xt_pool.tile([P, ND, NB], F8, tag="xt")
                    if c == 0:
                        # just-in-time startup feed: sync carries xt with odd
                        # weight tiles interleaved, gpsimd carries even tiles;
                        # nothing rides the scalar queue (DMAs occupy the
                        # trigger engine's track and would delay evacs)
                        def wkv_dma(eng, jd, half):
                            eng.dma_start(
                                out=wkv_sb[:, jd, half * DM:(half + 1) * DM],
                                in_=wkv_in[jd * P:(jd + 1) * P,
                                           half * DM:(half + 1) * DM],
                            )

                        for jd in range(0, ND, 2):
                            wkv_dma(nc.gpsimd, jd, 0)
                        for jd in range(ND):
                            nc.sync.dma_start(
                                out=xt_t[:, jd, :],
                                in_=xt_in[jd * P:(jd + 1) * P,
                                          c * NB:(c + 1) * NB],
                            )
                            if jd % 2 == 0:
                                wkv_dma(nc.sync, jd + 1, 0)
                        for jd in range(0, ND, 2):
                            wkv_dma(nc.gpsimd, jd, 1)
                        for jd in range(1, ND, 2):
                            wkv_dma(nc.sync, jd, 1)
                        for jd in range(ND):
                            eng = nc.gpsimd if jd % 2 == 0 else nc.sync
                            eng.dma_start(
                                out=wq_sb[:, jd, :],
                                in_=wq_in[jd * P:(jd + 1) * P, :],
                            )
                        for jd in range(ND):
                            eng = nc.gpsimd if jd % 2 == 0 else nc.sync
                            eng.dma_start(
                                out=wlin_sb[:, jd, :],
                                in_=wlin_in[jd * P:(jd + 1) * P, :],
                            )
                    else:
                        load_xt(c, xt_t)

                    # kv projection (fp8 DoubleRow, K=256 per matmul)
                    ek_tiles = [None] * 4
                    v_tiles = [None] * 4
                    for t in range(4):
                        ek_tiles[t] = ek_pool.tile(
                            [P, DM], F8, tag=f"ek{c % 2}_{t}",
                            name=f"ek{c}_{t}"
                        )
                        v_tiles[t] = vt_pool.tile(
                            [P, NJ, P + 1], F8, tag=f"v{c % 2}_{t}",
                            name=f"vt{c}_{t}"
                        )
                    # k-half for all t first (weights stream k-cols first),
                    # then v-half; v evacs ride DVE to offload ScalarE
                    for half in range(2):
                        for n in range(2):
                            for t in range(4):
                                kvps = kvp_pool.tile([P, NB], F32, tag="kvp")
                                for jp in range(NDP):
                                    nc.tensor.matmul(
                                        kvps,
                                        xt_t[:, 2 * jp:2 * jp + 2,
                                             t * P:(t + 1) * P],
                                        wkv_sb[:, 2 * jp:2 * jp + 2,
                                               half * DM + n * NB:
                                               half * DM + (n + 1) * NB],
                                        start=(jp == 0),
                                        stop=(jp == NDP - 1),
                                        perf_mode=DR,
                                    )
                                if half == 0:
                                    nc.scalar.activation(
                                        ek_tiles[t][:, n * NB:(n + 1) * NB],
                                        kvps, AF.Exp, scale=1.0 / 128,
                                    )
                                else:
                                    nc.vector.tensor_scalar_mul(
                                        out=v_tiles[t][:, 4 * n:4 * n + 4, 0:P],
                                        in0=kvps.rearrange(
                                            "p (j e) -> p j e", j=4),
                                        scalar1=1.0 / 128,
                                    )

                    def ctx_block():
                        # ctx+colsum accumulate in PSUM across all chunks
                        for j in range(NJ):
                            cg, base = ctx_region(j)
                            for t in range(4):
                                # start=True clears has_written BANK-wide on
                                # HW, so only the bank's first matmul may
                                # carry it; co-tenant regions overwrite on
                                # virgin has_written instead
                                nc.tensor.matmul(
                                    cg[:, base:base + P + 1],
                                    ek_tiles[t][:, j * P:(j + 1) * P],
                                    v_tiles[t][:, j, 0:P + 1],
                                    start=False,
                                    stop=False,
                                    skip_group_check=True,
                                )

                    if c == sc - 1:
                        ctx_block()
                        # finalize inline, ahead of q/rowsum so their DVE
                        # backlog doesn't delay W28 (phase B's input)
                        bsrc_tiles = [None] * NJ
                        for j in range(NJ):
                            cg, base = ctx_region(j)
                            bsrc = consts.tile(
                                [P, P], F16, tag=f"bsrc{j}", name=f"bsrc{j}"
                            )
                            nc.scalar.activation(
                                bsrc, cg[:, base:base + P], AF.Copy
                            )
                            bsrc_tiles[j] = bsrc
                            cs_sb = consts.tile(
                                [P, 1], F32, tag=f"cs{j}", name=f"cs{j}"
                            )
                            nc.scalar.activation(
                                cs_sb, cg[:, base + P:base + P + 1], AF.Copy
                            )
                            rcs = consts.tile([P, 1], F32, tag=f"rcs{j}")
                            nc.vector.reciprocal_approx_fast(
                                out=rcs, in_=cs_sb
                            )
                            rcs_tiles[j] = rcs
                        for j in range(NJ):
                            trps = qp_pool.tile([P, NB], F32, tag="qp")
                            nc.tensor.matmul(
                                trps[:, 0:P], bsrc_tiles[j], ident
                            )
                            for db in range(2):
                                o = 64 * db
                                nc.vector.tensor_copy(
                                    bdt_tiles[j][o:o + 64, o:o + 64],
                                    trps[o:o + 64, o:o + 64],
                                )
                        for j in range(NJ):
                            for n in range(2):
                                w2ps = qp_pool.tile([P, NB], F32, tag="qp")
                                nc.tensor.matmul(
                                    w2ps,
                                    bdt_tiles[j],
                                    wlin_sb[:, j, n * NB:(n + 1) * NB],
                                )
                                nc.vector.tensor_scalar(
                                    out=w28_sb[:, j, n * NB:(n + 1) * NB],
                                    in0=w2ps,
                                    scalar1=rcs_tiles[j],
                                    scalar2=SCALE * float(2.0 ** C),
                                    op0=mybir.AluOpType.mult,
                                    op1=mybir.AluOpType.mult,
                                )

                    # q projection (fp8 DoubleRow) -> eq16
                    e8_t = e8_pool.tile([P, NJ, NB], F8, tag=f"e8_{c}")
                    e8_res[c] = e8_t
                    eq16_tiles = [None] * NJ
                    for j in range(NJ):
                        qps = qp_pool.tile([P, NB], F32, tag="qp")
                        for jp in range(NDP):
                            nc.tensor.matmul(
                                qps,
                                wq_sb[:, 2 * jp:2 * jp + 2, j * P:(j + 1) * P],
                                xt_t[:, 2 * jp:2 * jp + 2, :],
                                start=(jp == 0),
                                stop=(jp == NDP - 1),
                                perf_mode=DR,
                            )
                        eq16 = eq_pool.tile([P, NB], F16, tag=f"eq{j}")
                        nc.scalar.activation(
                            eq16, qps, AF.Exp,
                            scale=1.0 / 128, bias=cstq_sb[:, j:j + 1],
                        )
                        eq16_tiles[j] = eq16

                    # rowsum (1/64-blockones matmul) -> rr -> eq8 = eq16*rr
                    # all recips issue before the muls so PSUM frees early
                    # (phase B's yp banks wait on the last rsps reader);
                    # chunk sc-1 parks rsps in the spare 8th bank
                    rr_tiles = [None] * NJ
                    for j in range(NJ):
                        pool = rsp7_pool if c == sc - 1 else qp_pool
                        rsps = pool.tile(
                            [P, NB], F32,
                            tag="rsp7" if c == sc - 1 else "qp",
                            name=f"rsps{c}_{j}",
                        )
                        nc.tensor.matmul(rsps, blkones, eq16_tiles[j])
                        rr = rr_pool.tile([P, NB], F32, tag=f"rr{j}",
                                          name=f"rr{c}_{j}")
                        nc.vector.reciprocal_approx_fast(out=rr, in_=rsps)
                        rr_tiles[j] = rr
                    for j in range(NJ):
                        nc.vector.tensor_mul(
                            e8_t[:, j, :], eq16_tiles[j], rr_tiles[j]
                        )

                    if c < sc - 1:
                        ctx_block()

            y_pool = stk.enter_context(tc.tile_pool(name="ysb", bufs=3))

            # ---------------- phase B: y = eq8.T @ W28 (fp8 DoubleRow)
            with tc.tile_pool(name="yp", bufs=3, space="PSUM") as yp_pool:
                for c in range(sc):
                    for t in range(4):
                        yps = yp_pool.tile([P, DM], F32, tag="yp")
                        for n in range(2):
                            for jp in range(NJ // 2):
                                nc.tensor.matmul(
                                    yps[:, n * NB:(n + 1) * NB],
                                    e8_res[c][:, 2 * jp:2 * jp + 2,
                                              t * P:(t + 1) * P],
                                    w28_sb[:, 2 * jp:2 * jp + 2,
                                           n * NB:(n + 1) * NB],
                                    start=(jp == 0),
                                    stop=(jp == NJ // 2 - 1),
                                    perf_mode=DR,
                                )
                        ysb = y_pool.tile([P, DM], F16, tag="ysb")
                        for hn in range(2):
                            sl = slice(hn * NB, (hn + 1) * NB)
                            if (t + hn) % 2 == 0:
                                nc.vector.tensor_scalar_mul(
                                    out=ysb[:, sl], in0=yps[:, sl],
                                    scalar1=float(2.0 ** -(6 + C)),
                                )
                            else:
                                nc.scalar.activation(
                                    ysb[:, sl], yps[:, sl], AF.Copy,
                                    scale=float(2.0 ** -(6 + C)),
                                )
                            nc.sync.dma_start(
                                out=y_out[c * NB + t * P:
                                          c * NB + (t + 1) * P, sl],
                                in_=ysb[:, sl],
                            )
    nc.compile()
    return nc


def prepare_inputs(x, Wq, Wkv, Wlin, blin):
    """Host-side quantization/layout. Returns (in_maps, host_const[DM])."""
    import ml_dtypes

    F8NP = ml_dtypes.float8_e4m3
    x = np.asarray(x, dtype=np.float32)
    Wq = np.asarray(Wq, dtype=np.float32)
    Wkv = np.asarray(Wkv, dtype=np.float32)
    Wlin = np.asarray(Wlin, dtype=np.float32)
    blin = np.asarray(blin, dtype=np.float32).reshape(DM)

    b = x.shape[0]
    # centered, transposed x: [B, D, S] fp8
    xt8 = np.ascontiguousarray(
        (2.0 * x - 1.0).transpose(0, 2, 1)).astype(F8NP)
    wq8 = (64.0 * Wq).astype(F8NP)
    # permute Wkv columns: k-cols of all heads first, then v-cols
    wkv3 = Wkv.reshape(D, H, 2 * DH)
    wkv_perm = np.concatenate(
        [wkv3[:, :, :DH].reshape(D, DM), wkv3[:, :, DH:].reshape(D, DM)],
        axis=1,
    )
    wkv8 = (64.0 * wkv_perm).astype(F8NP)
    wlin16 = Wlin.astype(np.float16)

    # exact fp32 consts
    cst_q = 0.5 * Wq.sum(axis=0)                      # [DM]
    cstq_dev = np.ascontiguousarray(
        cst_q.reshape(NJ, P).T).astype(np.float32)    # [P, NJ]
    cst_v = 0.5 * wkv_perm[:, DM:].sum(axis=0)        # [DM] (v-col order = e)
    kappa = SCALE * (cst_v.astype(np.float64)
                     @ wlin16.astype(np.float64))     # [DM]
    host_const = (kappa + blin.astype(np.float64)).astype(np.float32)

    in_maps = [
        {
            "xt": xt8[i],
            "Wq": wq8,
            "Wkv": wkv8,
            "Wlin": wlin16,
            "cstq": cstq_dev,
        }
        for i in range(b)
    ]
    return in_maps, host_const


def finish_output(results, host_const, b):
    """Assemble full y from per-core y16 + host consts."""
    ys = []
    for i in range(b):
        y16 = np.asarray(results[i]["y"]).astype(np.float32)
        ys.append(y16 + host_const[None, :])
    return np.stack(ys)


def kernel(x, Wq, Wkv, Wlin, blin):
    from concourse.bass_utils import run_bass_kernel_spmd

    x = np.asarray(x, dtype=np.float32)
    b = x.shape[0]
    nc = build_nc(x.shape[1])
    in_maps, host_const = prepare_inputs(x, Wq, Wkv, Wlin, blin)
    res = run_bass_kernel_spmd(nc, in_maps, list(range(b)))
    return finish_output(res.results, host_const, b)


if __name__ == "__main__":
    rng = np.random.default_rng(0)
    x = rng.random((B, S, D), dtype=np.float32)
    Wq = (rng.standard_normal((D, DM)) * 0.02).astype(np.float32)
    Wkv = (rng.standard_normal((D, 2 * DM)) * 0.02).astype(np.float32)
    Wlin = (rng.standard_normal((DM, DM)) * 0.02).astype(np.float32)
    blin = np.zeros((DM,), dtype=np.float32)
    y = kernel(x=x, Wq=Wq, Wkv=Wkv, Wlin=Wlin, blin=blin)
    print(y.shape, y.dtype)
